# revision 1
# baseline (speedup 1.0000x reference)
"""Multi-head attention (B=4, N=2048, DIM=1024, H=16, DH=64) on 8 trn2 cores.

Sharding: data-parallel over batch (4) x tensor-parallel over heads (2 groups
of 8). Each core computes q/k/v projections for its 8 heads, attention, and a
partial output projection; the host sums the two partials per batch and adds
the bias.

Layout (per core):
  - x^T produced on-chip via PE transposes; q^T/k^T kept transposed
    [inner, tok] so scores^T = k^T_tile.T @ q^T (contract DH=64) needs no
    transposes; v natural [tok, inner] with an appended ones column so
    attn@v (out^T = v_aug.T @ exp^T) yields softmax denominators for free
    in row 64; exp on ScalarE with 1/sqrt(dh) folded into the activation
    scale (max-subtraction skipped: |scores| < ~5 for this distribution).
    Normalization = DVE reciprocal + gpsimd partition-broadcast + DVE mult;
    the normalized transposed output feeds the Wo matmul directly as lhsT.
  - All matmuls in float32r (~1.5e-4 rel err, 4x faster than fp32).

Schedule (engine queues execute in program order, so emission order is the
schedule):
  - Phase A pipelines token-block tb's PE transposes (DMA-paced) with
    tb-1's projection matmul groups. The last block's q-projection is
    deferred into phase B as spread single-matmul filler.
  - Phase B emits per (head, query-block) "units": 2 score matmuls + exp +
    the same head's attn@v pair lagged 3 units + at most one filler matmul
    (previous block's Wo projection, or deferred q-projection). ScalarE
    receives a new score group every ~1.1us and stays saturated; the PE
    fills the rest of each unit with exp-independent work.
"""
import numpy as np

import concourse.bass as bass
import concourse.mybir as mybir
import concourse.tile as tile
from concourse import bacc
from concourse.bass_utils import run_bass_kernel_spmd
from concourse.masks import make_identity

f32 = mybir.dt.float32
f32r = mybir.dt.float32r
AF = mybir.ActivationFunctionType

N = 2048          # tokens
DIM = 1024        # model dim
NHL = 8           # heads per core
DH = 64           # head dim
INNER = NHL * DH  # 512 per-core inner dim
SCALE = DH ** -0.5
TB = 512          # token block (phase A)
QB = 512          # query block (phase B)
NTB = N // TB     # 4
NQB = N // QB     # 4
NKT = N // 128    # 16 k-tiles
NDC = DIM // 128  # 8 dim chunks
NM = INNER // 128 # 4 inner chunks
NG = NKT // 2     # 8 kt-pair groups per block

OPTS = dict(
    ps_s_bufs=2,
    big_bufs=16,
    wring_bufs=7,
    attnp_bufs=2,
    smallp_bufs=1,
    xin_bufs=2,
    outp_bufs=2,
    av_lag=3,
    defer_q3=True,
    take_pat=(3, 3, 3, 3),
)


def build_nc(**over):
    o = dict(OPTS)
    o.update(over)

    nc = bacc.Bacc(None, target_bir_lowering=False)

    x_d = nc.dram_tensor("x", [N, DIM], f32, kind="ExternalInput")
    wq_d = nc.dram_tensor("wq", [DIM, INNER], f32r, kind="ExternalInput")
    wk_d = nc.dram_tensor("wk", [DIM, INNER], f32r, kind="ExternalInput")
    wv_d = nc.dram_tensor("wv", [DIM, INNER], f32r, kind="ExternalInput")
    wo_d = nc.dram_tensor("wo", [INNER, DIM], f32r, kind="ExternalInput")
    out_d = nc.dram_tensor("out", [N, DIM], f32, kind="ExternalOutput")

    wq_v = wq_d.rearrange("(c k) n -> k c n", k=128)
    wk_v = wk_d.rearrange("(c k) n -> k c n", k=128)
    wv_v = wv_d.rearrange("(c k) n -> k c n", k=128)
    wo_v = wo_d.rearrange("(c k) n -> k c n", k=128)

    with tile.TileContext(nc) as tc:
        with (
            tc.tile_pool(name="consts", bufs=1) as consts,
            tc.tile_pool(name="xin", bufs=o["xin_bufs"]) as xin,
            tc.tile_pool(name="wring", bufs=o["wring_bufs"]) as wring,
            tc.tile_pool(name="wop", bufs=1) as wop,
            tc.tile_pool(name="big", bufs=o["big_bufs"]) as big,
            tc.tile_pool(name="ktp", bufs=1) as ktp,
            tc.tile_pool(name="vp", bufs=1) as vp,
            tc.tile_pool(name="attnp", bufs=o["attnp_bufs"]) as attnp,
            tc.tile_pool(name="outp", bufs=o["outp_bufs"]) as outp,
            tc.tile_pool(name="smallp", bufs=o["smallp_bufs"]) as smallp,
            tc.tile_pool(name="ps_s", bufs=o["ps_s_bufs"], space="PSUM") as ps_s,
            tc.tile_pool(name="ps_o", bufs=2, space="PSUM") as ps_o,
            tc.tile_pool(name="ps_f", bufs=2, space="PSUM") as ps_f,
        ):
            ident = consts.tile([128, 128], f32)
            make_identity(nc, ident)

            kT = ktp.tile([128, NM, N], f32r)              # k^T [inner, tok]
            v_sb = vp.tile([128, NKT, NHL, DH + 1], f32r)  # v + ones col

            ones_sb = consts.tile([128, NKT, NHL], f32)
            nc.vector.memset(ones_sb, 1.0)
            nc.vector.tensor_copy(v_sb[:, :, :, DH], ones_sb)

            wo_sb = wop.tile([128, NM, DIM], f32r)

            # ---------------- Phase A ----------------
            qT_slots = {}

            def emit_transpose_unit(tb, ts, xT):
                x_sub = xin.tile([128, DIM], f32, name=f"x{tb}{ts}",
                                 tag="xin")
                r0 = tb * TB + ts * 128
                if tb == 0 and ts == 0:
                    # split the very first load per column chunk so the
                    # first transposes start as soon as 64KB lands
                    for dc in range(NDC):
                        nc.sync.dma_start(
                            x_sub[:, dc * 128:dc * 128 + 128],
                            x_d[r0:r0 + 128, dc * 128:dc * 128 + 128])
                else:
                    nc.sync.dma_start(x_sub, x_d[r0:r0 + 128, :])
                for dc in range(NDC):
                    pt = ps_o.tile([128, 128], f32, name=f"pt{dc}", tag="o")
                    nc.tensor.transpose(
                        pt, x_sub[:, dc * 128:dc * 128 + 128], ident)
                    nc.vector.tensor_copy(
                        xT[dc // 2][:, dc % 2, ts * 128:ts * 128 + 128], pt)

            def q_mms(tb, xT, m):
                """Thunks: 8 matmuls of one q^T group + evac on the last."""
                psq = ps_f.tile([128, TB], f32, name=f"psq{tb}{m}", tag="f")
                wq_s = wq_state[tb]

                def mm(dc):
                    nc.tensor.matmul(
                        psq,
                        wq_s[dc // 2][:, dc % 2, m * 128:m * 128 + 128],
                        xT[dc // 2][:, dc % 2, :],
                        start=(dc == 0), stop=(dc == NDC - 1))
                    if dc == NDC - 1:
                        jm = m // 2
                        if (tb, jm) not in qT_slots:
                            qT_slots[(tb, jm)] = big.tile(
                                [128, 2, QB], f32r, name=f"qT{tb}{jm}",
                                tag="big")
                        nc.vector.tensor_copy(
                            qT_slots[(tb, jm)][:, m % 2, :], psq)
                return [(lambda dc=dc: mm(dc)) for dc in range(NDC)]

            wq_state = {}
            v_state = {}

            def proj_groups(tb, xT, include_q, include_v=True):
                """Generator of group-emitting thunks (k, v[, q] order)."""
                wk_s = [wring.tile([128, 2, INNER], f32r, name=f"wk{tb}{j}",
                                   tag="wr") for j in range(NDC // 2)]
                for j in range(NDC // 2):
                    nc.sync.dma_start(wk_s[j], wk_v[:, 2 * j:2 * j + 2, :])

                def k_group(m):
                    psk = ps_f.tile([128, TB], f32, name=f"psk{m}", tag="f")
                    for dc in range(NDC):
                        nc.tensor.matmul(
                            psk,
                            wk_s[dc // 2][:, dc % 2, m * 128:m * 128 + 128],
                            xT[dc // 2][:, dc % 2, :],
                            start=(dc == 0), stop=(dc == NDC - 1))
                    nc.vector.tensor_copy(kT[:, m, tb * TB:tb * TB + TB],
                                          psk)

                for m in range(NM):
                    yield (lambda m=m: k_group(m))

                wv_s = [wring.tile([128, 2, INNER], f32r, name=f"wv{tb}{j}",
                                   tag="wr") for j in range(NDC // 2)]
                for j in range(NDC // 2):
                    nc.sync.dma_start(wv_s[j], wv_v[:, 2 * j:2 * j + 2, :])

                def v_mms(ts):
                    psv = ps_f.tile([128, TB], f32, name=f"psv{tb}{ts}",
                                    tag="f")

                    def mm(dc):
                        nc.tensor.matmul(
                            psv,
                            xT[dc // 2][:, dc % 2, ts * 128:ts * 128 + 128],
                            wv_s[dc // 2][:, dc % 2, :],
                            start=(dc == 0), stop=(dc == NDC - 1))
                        if dc == NDC - 1:
                            kt = tb * (TB // 128) + ts
                            nc.vector.tensor_copy(
                                v_sb[:, kt, :, 0:DH],
                                psv.rearrange("p (h d) -> p h d", h=NHL))
                    return [(lambda dc=dc: mm(dc)) for dc in range(NDC)]

                v_state[tb] = v_mms
                if include_v:
                    for ts in range(TB // 128):
                        yield (lambda ts=ts: [t() for t in v_mms(ts)])

                wq_s = [wring.tile([128, 2, INNER], f32r, name=f"wq{tb}{j}",
                                   tag="wr") for j in range(NDC // 2)]
                for j in range(NDC // 2):
                    nc.sync.dma_start(wq_s[j], wq_v[:, 2 * j:2 * j + 2, :])
                wq_state[tb] = wq_s
                if include_q:
                    for m in range(NM):
                        yield (lambda m=m: [t() for t in q_mms(tb, xT, m)])

            xTs = {}
            prev_groups = None
            for tb in range(NTB):
                xT = [big.tile([128, 2, TB], f32r, name=f"xT{tb}{j}",
                               tag="big") for j in range(NDC // 2)]
                xTs[tb] = xT
                for ts in range(TB // 128):
                    emit_transpose_unit(tb, ts, xT)
                    if prev_groups is not None:
                        for _ in range(o["take_pat"][ts]):
                            next(prev_groups)()
                last = tb == NTB - 1
                defer = last and o["defer_q3"]
                prev_groups = proj_groups(tb, xT, include_q=not defer,
                                          include_v=True)
                if last:
                    for g in prev_groups:
                        g()

            nc.sync.dma_start(wo_sb, wo_v)

            # filler: single-matmul thunks consumed one per unit in phase B
            filler = []
            if o["defer_q3"]:
                for m in range(NM):
                    filler.extend(q_mms(NTB - 1, xTs[NTB - 1], m))

            # ---------------- Phase B ----------------
            def wo_unit_mms(qb, attnT, u):
                qs, d = u // 2, u % 2
                psf = ps_f.tile([128, 512], f32, name=f"psf{qs}{d}",
                                tag="f")

                def mm(m):
                    nc.tensor.matmul(
                        psf,
                        attnT[:, m, qs * 128:qs * 128 + 128],
                        wo_sb[:, m, d * 512:d * 512 + 512],
                        start=(m == 0), stop=(m == NM - 1))
                    if m == NM - 1:
                        osb = outp.tile([128, 512], f32, name=f"osb{qs}{d}",
                                        tag="osb")
                        nc.vector.tensor_copy(osb, psf)
                        r0 = qb * QB + qs * 128
                        nc.sync.dma_start(
                            out_d[r0:r0 + 128, d * 512:d * 512 + 512], osb)
                return [(lambda m=m: mm(m)) for m in range(NM)]

            def emit_av(h, pso, expT, g):
                for i in range(2):
                    kt = 2 * g + i
                    nc.tensor.matmul(
                        pso, v_sb[:, kt, h, :], expT[g][:, i, :],
                        start=(kt == 0), stop=(kt == NKT - 1))

            def emit_norm(h, pso, attnT):
                po = h % 2 * 64
                recip = smallp.tile([1, QB], f32, name=f"recip{h}",
                                    tag="recip")
                nc.vector.reciprocal(recip, pso[DH:DH + 1, :])
                bcast = smallp.tile([64, QB], f32, name=f"bcast{h}",
                                    tag="bcast")
                nc.gpsimd.partition_broadcast(bcast, recip)
                nc.vector.tensor_mul(attnT[po:po + 64, h // 2, :],
                                     pso[0:DH, :], bcast)

            lag = o["av_lag"]
            av_q = []   # (h, pso, et, g, is_last, attnT, qb)

            def deq():
                h, pso, et, g, is_last, attnT_, _qb = av_q.pop(0)
                for i in range(2):
                    kt = 2 * g + i
                    nc.tensor.matmul(
                        pso, v_sb[:, kt, h, :], et[:, i, :],
                        start=(kt == 0), stop=(kt == NKT - 1))
                if is_last:
                    emit_norm(h, pso, attnT_)

            for qb in range(NQB):
                attnT = attnp.tile([128, NM, QB], f32r, name=f"attnT{qb}",
                                   tag="attnT")
                for h in range(NHL):
                    po = h % 2 * 64
                    jm_q = (h // 2) // 2
                    im_q = (h // 2) % 2
                    qs_t = qT_slots[(qb, jm_q)]
                    pso = ps_o.tile([DH + 1, QB], f32, name=f"pso{h}",
                                    tag="o")
                    for g in range(NG):
                        pss = ps_s.tile([128, 2, QB], f32, name=f"pss{g}",
                                        tag="s")
                        for i in range(2):
                            kt = 2 * g + i
                            nc.tensor.matmul(
                                pss[:, i, :],
                                kT[po:po + 64, h // 2,
                                   kt * 128:kt * 128 + 128],
                                qs_t[po:po + 64, im_q, :],
                                start=True, stop=True)
                        et = big.tile([128, 2, QB], f32r, name=f"eT{h}{g}",
                                      tag="big")
                        nc.scalar.activation(out=et, in_=pss, func=AF.Exp,
                                             scale=SCALE)
                        av_q.append((h, pso, et, g, g == NG - 1, attnT, qb))
                        if len(av_q) > lag:
                            deq()
                        if filler:
                            filler.pop(0)()
                            if len(filler) > 32:
                                filler.pop(0)()

                if qb + 1 < NQB:
                    for u in range(8):
                        filler.extend(wo_unit_mms(qb, attnT, u))

            while av_q:
                deq()
            for u in range(8):
                for t in wo_unit_mms(NQB - 1, attnT, u):
                    t()

    nc.compile()
    return nc


_NC = None


def _get_nc():
    global _NC
    if _NC is None:
        _NC = build_nc()
    return _NC


def kernel(x, Wq, Wk, Wv, Wo, bo):
    x = np.ascontiguousarray(np.asarray(x, dtype=np.float32))
    Wq = np.asarray(Wq, dtype=np.float32)
    Wk = np.asarray(Wk, dtype=np.float32)
    Wv = np.asarray(Wv, dtype=np.float32)
    Wo = np.asarray(Wo, dtype=np.float32)
    bo = np.asarray(bo, dtype=np.float32)

    B = x.shape[0]
    nc = _get_nc()
    in_maps = []
    for c in range(8):
        b, hh = c // 2, c % 2
        sl = slice(hh * INNER, hh * INNER + INNER)
        in_maps.append({
            "x": np.ascontiguousarray(x[b]),
            "wq": np.ascontiguousarray(Wq[:, sl]),
            "wk": np.ascontiguousarray(Wk[:, sl]),
            "wv": np.ascontiguousarray(Wv[:, sl]),
            "wo": np.ascontiguousarray(Wo[sl, :]),
        })
    res = run_bass_kernel_spmd(nc, in_maps, core_ids=list(range(8)))
    out = np.empty((B, N, DIM), dtype=np.float32)
    for b in range(B):
        out[b] = res.results[2 * b]["out"] + res.results[2 * b + 1]["out"] + bo
    return out



# revision 37
# speedup vs baseline: 1.2266x; 1.2266x over previous
"""Multi-head attention (B=4, N=2048, DIM=1024, H=16, DH=64) on 8 trn2 cores.

Sharding: data-parallel over batch (4) x tensor-parallel over heads (2 groups
of 8). Each core computes q/k/v projections for its 8 heads, attention, and a
partial output projection; the host sums the two partials per batch and adds
the bias.

Key design points (v2):
  - Host passes x and all weights in bf16 (halves DMA); weights are resident
    in SBUF (loaded once, not per token-block).
  - Scores run as fp8e4 DoubleRow matmuls (0.5 cycles/row): q^T/k^T are
    quantized to fp8 at PSUM evacuation. The DH=64 contraction uses
    sub-tile 0; sub-tile 1 multiplies a zeroed q block against don't-care k
    columns, so no partition-split layout is needed.
  - attn@v uses the swapped orientation: lhsT = exp tile [keys, 128 queries],
    rhs = v_aug [keys, 65] (ones column -> softmax denominators), giving
    65-row matmuls (4x fewer moving rows than the [65, q] orientation).
    Outputs land naturally as [queries, 65]; normalization is a per-partition
    reciprocal+scale, then a PE transpose rebuilds attn^T for the Wo matmul.
  - exp on ScalarE (the hard floor: ~267us for 33.5M elements) is kept
    saturated: phase A only computes the first unit's dependencies
    (transposes, k[m0], q[tb0,m0]); all other projections are deadline-
    ordered filler inside phase B.
  - Unit order is h-outer (m-chunk deps arrive just in time); the last two
    head-rows interleave qb so output-projection work spreads before the tail.
"""
import numpy as np
import ml_dtypes

import concourse.bass as bass
import concourse.mybir as mybir
import concourse.tile as tile
from concourse import bacc
from concourse.bass_utils import run_bass_kernel_spmd
from concourse.masks import make_identity

f32 = mybir.dt.float32
bf16 = mybir.dt.bfloat16
f8 = mybir.dt.float8e4
AF = mybir.ActivationFunctionType
DR = mybir.MatmulPerfMode.DoubleRow

N = 2048          # tokens
DIM = 1024        # model dim
NHL = 8           # heads per core
DH = 64           # head dim
INNER = NHL * DH  # 512 per-core inner dim
SCALE = DH ** -0.5
TB = 512          # token block
NTB = N // TB     # 4
NKT = N // 128    # 16 k-tiles
NDC = DIM // 128  # 8 dim chunks
NM = INNER // 128 # 4 inner chunks (head pairs)
NQB = 4           # query blocks (= NTB)
QB = 512

OPTS = dict(
    et_bufs=16,
    fill_budget=520.0,
    stage_bufs=6,
)


def build_nc(**over):
    o = dict(OPTS)
    o.update(over)

    nc = bacc.Bacc(None, target_bir_lowering=False)

    x_d = nc.dram_tensor("x", [N, DIM], bf16, kind="ExternalInput")
    wq_d = nc.dram_tensor("wq", [DIM, INNER], bf16, kind="ExternalInput")
    wk_d = nc.dram_tensor("wk", [DIM, INNER], bf16, kind="ExternalInput")
    wv_d = nc.dram_tensor("wv", [DIM, INNER], bf16, kind="ExternalInput")
    wo_d = nc.dram_tensor("wo", [INNER, DIM], bf16, kind="ExternalInput")
    out_d = nc.dram_tensor("out", [N, DIM], f32, kind="ExternalOutput")

    # x rows r = tb*512 + t*128 + p  ->  [tb][p, t, d]
    x_v = x_d.rearrange("(b t p) d -> b p t d", b=NTB, t=TB // 128)
    wq_v = wq_d.rearrange("(c k) n -> k c n", k=128)   # [128, 8, 512]
    wk_v = wk_d.rearrange("(c k) n -> k c n", k=128)
    wv_v = wv_d.rearrange("(c k) n -> k c n", k=128)
    wo_v = wo_d.rearrange("(c k) n -> k c n", k=128)   # [128, 4, 1024]

    with tile.TileContext(nc) as tc:
        with (
            tc.tile_pool(name="consts", bufs=1) as consts,
            tc.tile_pool(name="xin", bufs=3) as xin,
            tc.tile_pool(name="wsb", bufs=1) as wsb,
            tc.tile_pool(name="xTp", bufs=4) as xTp,
            tc.tile_pool(name="kqv", bufs=1) as kqv,
            tc.tile_pool(name="etp", bufs=o["et_bufs"]) as etp,
            tc.tile_pool(name="stp", bufs=o["stage_bufs"]) as stp,
            tc.tile_pool(name="attnp", bufs=4) as attnp,
            tc.tile_pool(name="outp", bufs=2) as outp,
            tc.tile_pool(name="ps_s", bufs=2, space="PSUM") as ps_s,
            tc.tile_pool(name="ps_av", bufs=2, space="PSUM") as ps_av,
            tc.tile_pool(name="ps_f", bufs=2, space="PSUM") as ps_f,
        ):
            ident = consts.tile([128, 128], bf16)
            make_identity(nc, ident)

            # preload the Exp activation table immediately
            dummy = consts.tile([128, 1], f32)
            nc.scalar.activation(out=dummy, in_=ident[:, 0:1], func=AF.Exp)

            # keep the PE p-state ramp warm until real work arrives
            # (cold/idle PE runs matmuls at 2-3.7x the cycle time)
            for _ in range(30):
                scratch = ps_s.tile([128, 128], bf16, name="warm", tag="s")
                nc.tensor.transpose(scratch, ident, ident)

            kT8 = kqv.tile([128, NM, N + 128], f8)           # fp8 k^T + pad
            qT8 = kqv.tile([128, NTB, NM, 2, QB], f8)        # fp8 q^T + zeros
            v_sb = kqv.tile([128, NKT, NHL, DH + 1], bf16)   # v + ones col

            # one-time zero/one fills on gpsimd (idle engine)
            nc.gpsimd.memset(v_sb[:, :, :, DH], 1.0)
            nc.gpsimd.memset(kT8[:, :, N:N + 128], 0.0)
            nc.gpsimd.memset(qT8[:, :, :, 1, :], 0.0)

            # ---- DMA: x in halves; weight m0 chunks early, rest later ----
            wk_sb = wsb.tile([128, NDC, INNER], bf16)
            wq_sb = wsb.tile([128, NDC, INNER], bf16)
            wv_sb = wsb.tile([128, NDC, INNER], bf16)
            wo_sb = wsb.tile([128, NM, DIM], bf16)
            x_subs = []
            for tb in range(NTB):
                x_subs.append(xin.tile([128, TB // 128, DIM], bf16,
                                       name=f"x{tb}", tag="xin"))

            def dma_x(tb, half):
                sl = slice(2 * half, 2 * half + 2)
                nc.sync.dma_start(x_subs[tb][:, sl, :], x_v[tb, :, sl, :])

            def dma_xq(tb, ts):
                nc.sync.dma_start(x_subs[tb][:, ts:ts + 1, :],
                                  x_v[tb, :, ts:ts + 1, :])

            def dma_w(sb_t, view, m0, m1):
                nc.sync.dma_start(sb_t[:, :, m0 * 128:m1 * 128],
                                  view[:, :, m0 * 128:m1 * 128])

            dma_xq(0, 0)
            dma_xq(0, 1)
            dma_w(wk_sb, wk_v, 0, 1)
            dma_xq(0, 2)
            dma_xq(0, 3)
            dma_w(wq_sb, wq_v, 0, 1)
            dma_x(1, 0)
            dma_x(1, 1)
            dma_w(wv_sb, wv_v, 0, 1)
            dma_x(2, 0)
            dma_x(2, 1)
            dma_x(3, 0)
            dma_x(3, 1)
            dma_w(wk_sb, wk_v, 1, 4)
            dma_w(wv_sb, wv_v, 1, 4)
            dma_w(wq_sb, wq_v, 1, 4)
            nc.sync.dma_start(wo_sb, wo_v)

            # ---------------- projection groups ----------------
            xTs = []

            def tb_chain(tb, with_q):
                """ts-major transposes (pipelining with the x DMA quarters,
                alternating psum pools to avoid buf WAR serialization),
                followed by the m0 k (and optionally q) projection."""
                xT = xTp.tile([128, NDC, TB], bf16, name=f"xT{tb}", tag="xT")
                xTs.append(xT)
                for ts in range(TB // 128):
                    pool, tg = (ps_av, "av") if ts % 2 == 0 else (ps_f, "f")
                    pt = pool.tile([128, NDC, 128], bf16, name="pt", tag=tg)
                    for dc in range(NDC):
                        nc.tensor.transpose(
                            pt[:, dc, :],
                            x_subs[tb][:, ts, dc * 128:dc * 128 + 128],
                            ident)
                    nc.vector.tensor_copy(
                        xT[:, :, ts * 128:ts * 128 + 128], pt)
                if not with_q:
                    k_group(tb, 0)
                    return
                psk = ps_f.tile([128, TB], f32, name="pskc", tag="f")
                psq = ps_f.tile([128, TB], f32, name="psqc", tag="f")
                for dc in range(NDC):
                    nc.tensor.matmul(psk, wk_sb[:, dc, 0:128],
                                     xT[:, dc, :],
                                     start=(dc == 0), stop=(dc == NDC - 1))
                    if dc == NDC - 1:
                        nc.vector.tensor_copy(
                            kT8[:, 0, tb * TB:tb * TB + TB], psk)
                    nc.tensor.matmul(psq, wq_sb[:, dc, 0:128],
                                     xT[:, dc, :],
                                     start=(dc == 0), stop=(dc == NDC - 1))
                    if dc == NDC - 1:
                        nc.vector.tensor_copy(qT8[:, tb, 0, 0, :], psq)

            def kq_steps(w_sb, tb, m, evac):
                """Two steps of 4 matmuls; evac(psum) runs on the last.
                The psum group stays open between steps, so the pop
                machinery must not emit other ps_f tiles in between."""
                cell = {}

                def half(h):
                    if h == 0:
                        cell["ps"] = ps_f.tile([128, TB], f32, name="pskq",
                                               tag="f")
                    for dc in range(4 * h, 4 * h + 4):
                        nc.tensor.matmul(
                            cell["ps"], w_sb[:, dc, m * 128:m * 128 + 128],
                            xTs[tb][:, dc, :],
                            start=(dc == 0), stop=(dc == NDC - 1))
                    if h == 1:
                        evac(cell["ps"])
                return [(880, lambda: half(0)), (1540, lambda: half(1))]

            def k_steps(tb, m):
                return kq_steps(
                    wk_sb, tb, m,
                    lambda ps: nc.vector.tensor_copy(
                        kT8[:, m, tb * TB:tb * TB + TB], ps))

            def q_steps(tb, m):
                return kq_steps(
                    wq_sb, tb, m,
                    lambda ps: nc.vector.tensor_copy(qT8[:, tb, m, 0, :],
                                                     ps))

            def k_group(tb, m):
                for _, fn in k_steps(tb, m):
                    fn()

            def q_group(tb, m):
                for _, fn in q_steps(tb, m):
                    fn()

            def v_group(tb, ts, h):
                """v for ONE head x 128 tokens; out natural [tok, 64]."""
                psv = ps_f.tile([128, DH], f32, name="psv", tag="f")
                for dc in range(NDC):
                    nc.tensor.matmul(
                        psv, xTs[tb][:, dc, ts * 128:ts * 128 + 128],
                        wv_sb[:, dc, h * DH:h * DH + DH],
                        start=(dc == 0), stop=(dc == NDC - 1))
                kt = tb * (TB // 128) + ts
                nc.vector.tensor_copy(v_sb[:, kt, h, 0:DH], psv)

            # ---------------- Phase A ----------------
            # tb0's chain first (incl. q) so scores(g0,g1) (keys of tb0)
            # start early; later tbs gate only later score groups via
            # subtile deps.
            for tb in range(NTB):
                tb_chain(tb, with_q=(tb == 0))

            # ---------------- filler (deadline order) ----------------
            # NOTE: pops bound EMISSION order; a consumer emitted before its
            # producer reads garbage (deps are tracked in emission order).
            # Entries are lists of (cost, fn) steps; an entry's steps may
            # span pop calls but no other entry interleaves (ps_f safety).
            filler = []
            fill_state = {"cur": None, "i": 0}

            def add_k(m):
                for tb in range(NTB):
                    filler.append(k_steps(tb, m))

            def add_q(tb, m):
                filler.append(q_steps(tb, m))

            def add_v(h):
                for tb in range(NTB):
                    for ts in range(TB // 128):
                        filler.append(
                            [(310, lambda tb=tb, ts=ts: v_group(tb, ts, h))])

            # unit order: head pairs sharing an m-chunk, m-major:
            #   (h0,qb0),(h1,qb0),(h0,qb1),... so consecutive units mostly
            #   share projections. Emission deadlines (8 slots per unit):
            #   scores(u) need k[m], qT8[qb, m] by u's slot 0;
            #   replay(u) at unit u+1 needs that head's v by then.
            add_v(0)                      # by u1 (boosted budget)
            add_v(1)                      # by u2
            for tb in (1, 2, 3):
                add_q(tb, 0)              # by u2/u4/u6
            for m in (1, 2, 3):
                add_k(m)                  # by u8m
                add_q(0, m)
                add_v(2 * m)
                add_q(1, m)
                add_v(2 * m + 1)
                add_q(2, m)
                add_q(3, m)

            def pop_filler(budget):
                spent = 0.0
                while spent < budget:
                    if fill_state["cur"] is None:
                        if not filler:
                            return
                        fill_state["cur"] = filler.pop(0)
                        fill_state["i"] = 0
                    steps = fill_state["cur"]
                    cost, fn = steps[fill_state["i"]]
                    fn()
                    spent += cost
                    fill_state["i"] += 1
                    if fill_state["i"] >= len(steps):
                        fill_state["cur"] = None

            # ---------------- Phase B ----------------
            units = [(2 * m + hh, qb)
                     for m in range(NM) for qb in range(NQB)
                     for hh in range(2)]

            attnT = [attnp.tile([128, NM, QB], bf16, name=f"attnT{qb}",
                                tag="attnT") for qb in range(NQB)]

            def wo_unit(qb, qs, d):
                psf = ps_f.tile([128, 512], f32, name=f"psf{qs}{d}", tag="f")
                for m in range(NM):
                    nc.tensor.matmul(
                        psf, attnT[qb][:, m, qs * 128:qs * 128 + 128],
                        wo_sb[:, m, d * 512:d * 512 + 512],
                        start=(m == 0), stop=(m == NM - 1))
                osb = wo_unit.osbs.get((qb, qs))
                if osb is None:
                    osb = outp.tile([128, DIM], f32, name=f"osb{qs}",
                                    tag="osb")
                    wo_unit.osbs[(qb, qs)] = osb
                nc.vector.tensor_copy(osb[:, d * 512:d * 512 + 512], psf)
                if d == 1:
                    r0 = qb * QB + qs * 128
                    nc.sync.dma_start(out_d[r0:r0 + 128, :], osb)
                    del wo_unit.osbs[(qb, qs)]
            wo_unit.osbs = {}

            def add_wo(qb):
                for qs in range(4):
                    for d in range(2):
                        filler.append(
                            [(880, lambda qs=qs, d=d: wo_unit(qb, qs, d))])

            # per-unit state
            ustate = {}   # u_idx -> dict(ets, psos, stages, h, qb)

            def emit_scores_exp(u_idx, g):
                h, qb = units[u_idx]
                po = (h % 2) * 64
                m = h // 2
                st = ustate[u_idx]
                pss = ps_s.tile([128, 2, QB], f32, name=f"pss{g}", tag="s")
                for i in range(2):
                    kt = 2 * g + i
                    c0 = kt * 128
                    nc.tensor.matmul(
                        pss[:, i, :],
                        kT8[po:po + 64, m, c0:c0 + 256].rearrange(
                            "p (s f) -> p s f", s=2),
                        qT8[po:po + 64, qb, m, :, :],
                        start=True, stop=True, perf_mode=DR)
                et = etp.tile([128, 2, QB], bf16, name=f"et{g}", tag="et")
                nc.scalar.activation(out=et, in_=pss, func=AF.Exp,
                                     scale=SCALE)
                st["ets"].append(et)

            def replay_qs(u_idx, qs, pool=None):
                """attn@v for one query sub-tile of a finished unit."""
                h, qb = units[u_idx]
                st = ustate[u_idx]
                pool = pool or ps_av
                pso = pool.tile([128, 512], f32, name=f"pso{qs}",
                                tag="av" if pool is ps_av else "s")
                st["psos"][qs] = pso
                for g in range(8):
                    et = st["ets"][g]
                    for i in range(2):
                        kt = 2 * g + i
                        nc.tensor.matmul(
                            pso[:, 0:DH + 1],
                            et[:, i, qs * 128:qs * 128 + 128],
                            v_sb[:, kt, h, :],
                            start=(kt == 0), stop=(kt == NKT - 1))

            def norm_qs(u_idx, qs):
                h, qb = units[u_idx]
                st = ustate[u_idx]
                pso = st["psos"][qs]
                recip = stp.tile([128, 1], f32, name=f"rc{qs}", tag="rc")
                nc.vector.reciprocal(recip, pso[:, DH:DH + 1])
                stage = stp.tile([128, DH], bf16, name=f"st{qs}", tag="st")
                nc.vector.tensor_scalar_mul(stage, pso[:, 0:DH], recip)
                st["stages"][qs] = stage

            def trans_qs(u_idx, qs):
                h, qb = units[u_idx]
                po = (h % 2) * 64
                m = h // 2
                st = ustate[u_idx]
                ptT = ps_av.tile([64, 128], bf16, name=f"ptT{qs}", tag="av")
                nc.tensor.transpose(ptT, st["stages"][qs], ident)
                nc.vector.tensor_copy(
                    attnT[qb][po:po + 64, m, qs * 128:qs * 128 + 128], ptT)

            def unit_post(u_idx, g):
                """Work for the previous unit, scheduled into slot g."""
                if u_idx < 0:
                    return
                h, qb = units[u_idx]
                if g == 0:
                    replay_qs(u_idx, 0)
                elif g == 1:
                    replay_qs(u_idx, 1)
                elif g == 2:
                    norm_qs(u_idx, 0)
                    replay_qs(u_idx, 2)
                elif g == 3:
                    norm_qs(u_idx, 1)
                    replay_qs(u_idx, 3)
                elif g == 4:
                    norm_qs(u_idx, 2)
                    trans_qs(u_idx, 0)
                elif g == 5:
                    norm_qs(u_idx, 3)
                    trans_qs(u_idx, 1)
                    trans_qs(u_idx, 2)
                elif g == 6:
                    trans_qs(u_idx, 3)
                    if h == 7:
                        add_wo(qb)
                    ustate.pop(u_idx, None)

            for u_idx in range(len(units)):
                ustate[u_idx] = dict(ets=[], psos={}, stages={})
                if u_idx < 2:
                    budget = 900.0
                elif u_idx >= 24:
                    budget = 700.0
                else:
                    budget = o["fill_budget"]
                for g in range(8):
                    emit_scores_exp(u_idx, g)
                    pop_filler(budget)
                    unit_post(u_idx - 1, g)

            # ---------------- tail (pipelined per query sub-tile) ----------
            last = len(units) - 1
            qb_last = units[last][1]
            replay_qs(last, 0)
            replay_qs(last, 1)
            replay_qs(last, 2, pool=ps_s)
            replay_qs(last, 3, pool=ps_s)
            for qs in range(4):
                norm_qs(last, qs)
                trans_qs(last, qs)
                wo_unit(qb_last, qs, 0)
                wo_unit(qb_last, qs, 1)
            while filler:
                pop_filler(1e9)

    nc.compile()
    return nc


_NC = None


def _get_nc():
    global _NC
    if _NC is None:
        _NC = build_nc()
    return _NC


def kernel(x, Wq, Wk, Wv, Wo, bo):
    x = np.asarray(x, dtype=np.float32)
    Wq = np.asarray(Wq, dtype=np.float32)
    Wk = np.asarray(Wk, dtype=np.float32)
    Wv = np.asarray(Wv, dtype=np.float32)
    Wo = np.asarray(Wo, dtype=np.float32)
    bo = np.asarray(bo, dtype=np.float32)

    B = x.shape[0]
    bf = ml_dtypes.bfloat16
    nc = _get_nc()
    in_maps = []
    for c in range(8):
        b, hh = c // 2, c % 2
        sl = slice(hh * INNER, hh * INNER + INNER)
        in_maps.append({
            "x": np.ascontiguousarray(x[b].astype(bf)),
            "wq": np.ascontiguousarray(Wq[:, sl].astype(bf)),
            "wk": np.ascontiguousarray(Wk[:, sl].astype(bf)),
            "wv": np.ascontiguousarray(Wv[:, sl].astype(bf)),
            "wo": np.ascontiguousarray(Wo[sl, :].astype(bf)),
        })
    res = run_bass_kernel_spmd(nc, in_maps, core_ids=list(range(8)))
    out = np.empty((B, N, DIM), dtype=np.float32)
    for b in range(B):
        out[b] = res.results[2 * b]["out"] + res.results[2 * b + 1]["out"] + bo
    return out


# revision 63
# speedup vs baseline: 1.2794x; 1.0430x over previous
"""Multi-head attention (B=4, N=2048, DIM=1024, H=16, DH=64) on 8 trn2 cores.

Sharding: data-parallel over batch (4) x tensor-parallel over heads (2 groups
of 8). Each core computes q/k/v projections for its 8 heads, attention, and a
partial output projection; the host sums the two partials per batch and adds
the bias.

Key design points (v2):
  - Host passes x and all weights in bf16 (halves DMA); weights are resident
    in SBUF (loaded once, not per token-block).
  - Scores run as fp8e4 DoubleRow matmuls (0.5 cycles/row): q^T/k^T are
    quantized to fp8 at PSUM evacuation. The DH=64 contraction uses
    sub-tile 0; sub-tile 1 multiplies a zeroed q block against don't-care k
    columns, so no partition-split layout is needed.
  - attn@v uses the swapped orientation: lhsT = exp tile [keys, 128 queries],
    rhs = v_aug [keys, 65] (ones column -> softmax denominators), giving
    65-row matmuls (4x fewer moving rows than the [65, q] orientation).
    Outputs land naturally as [queries, 65]; normalization is a per-partition
    reciprocal+scale, then a PE transpose rebuilds attn^T for the Wo matmul.
  - exp on ScalarE (the hard floor: ~267us for 33.5M elements) is kept
    saturated: phase A only computes the first unit's dependencies
    (transposes, k[m0], q[tb0,m0]); all other projections are deadline-
    ordered filler inside phase B.
  - Unit order is h-outer (m-chunk deps arrive just in time); the last two
    head-rows interleave qb so output-projection work spreads before the tail.
"""
import collections

import numpy as np
import ml_dtypes

import concourse.bass as bass
import concourse.mybir as mybir
import concourse.tile as tile
from concourse import bacc
from concourse.bass_utils import run_bass_kernel_spmd
from concourse.masks import make_identity

f32 = mybir.dt.float32
bf16 = mybir.dt.bfloat16
f8 = mybir.dt.float8e4
AF = mybir.ActivationFunctionType
DR = mybir.MatmulPerfMode.DoubleRow

N = 2048          # tokens
DIM = 1024        # model dim
NHL = 8           # heads per core
DH = 64           # head dim
INNER = NHL * DH  # 512 per-core inner dim
SCALE = DH ** -0.5
TB = 512          # token block
NTB = N // TB     # 4
NKT = N // 128    # 16 k-tiles
NDC = DIM // 128  # 8 dim chunks
NM = INNER // 128 # 4 inner chunks (head pairs)
NQB = 4           # query blocks (= NTB)
QB = 512

OPTS = dict(
    et_bufs=16,
    fill_budget=450.0,
    early_budget=800.0,
    late_budget=700.0,
    backlog_hi=9,
    stage_bufs=6,
)


def build_nc(**over):
    o = dict(OPTS)
    o.update(over)

    nc = bacc.Bacc(None, target_bir_lowering=False)

    x_d = nc.dram_tensor("x", [N, DIM], bf16, kind="ExternalInput")
    wq_d = nc.dram_tensor("wq", [DIM, INNER], bf16, kind="ExternalInput")
    wk_d = nc.dram_tensor("wk", [DIM, INNER], bf16, kind="ExternalInput")
    wv_d = nc.dram_tensor("wv", [DIM, INNER], bf16, kind="ExternalInput")
    wo_d = nc.dram_tensor("wo", [INNER, DIM], bf16, kind="ExternalInput")
    out_d = nc.dram_tensor("out", [N, DIM], f32, kind="ExternalOutput")

    # x rows r = tb*512 + t*128 + p  ->  [tb][p, t, d]
    x_v = x_d.rearrange("(b t p) d -> b p t d", b=NTB, t=TB // 128)
    wq_v = wq_d.rearrange("(c k) n -> k c n", k=128)   # [128, 8, 512]
    wk_v = wk_d.rearrange("(c k) n -> k c n", k=128)
    wv_v = wv_d.rearrange("(c k) n -> k c n", k=128)
    wo_v = wo_d.rearrange("(c k) n -> k c n", k=128)   # [128, 4, 1024]

    with tile.TileContext(nc) as tc:
        with (
            tc.tile_pool(name="consts", bufs=1) as consts,
            tc.tile_pool(name="xin", bufs=3) as xin,
            tc.tile_pool(name="wsb", bufs=1) as wsb,
            tc.tile_pool(name="xTp", bufs=4) as xTp,
            tc.tile_pool(name="kqv", bufs=1) as kqv,
            tc.tile_pool(name="etp", bufs=o["et_bufs"]) as etp,
            tc.tile_pool(name="stp", bufs=o["stage_bufs"]) as stp,
            tc.tile_pool(name="attnp", bufs=4) as attnp,
            tc.tile_pool(name="outp", bufs=2) as outp,
            tc.tile_pool(name="ps_s", bufs=2, space="PSUM") as ps_s,
            tc.tile_pool(name="ps_av", bufs=2, space="PSUM") as ps_av,
            tc.tile_pool(name="ps_f", bufs=2, space="PSUM") as ps_f,
        ):
            ident = consts.tile([128, 128], bf16)
            make_identity(nc, ident)

            # preload the Exp activation table immediately
            dummy = consts.tile([128, 1], f32)
            nc.scalar.activation(out=dummy, in_=ident[:, 0:1], func=AF.Exp)

            # keep the PE p-state ramp warm until real work arrives
            # (cold/idle PE runs matmuls at 2-3.7x the cycle time)
            for _ in range(30):
                scratch = ps_s.tile([128, 128], bf16, name="warm", tag="s")
                nc.tensor.transpose(scratch, ident, ident)

            kT8 = kqv.tile([128, NM, NTB, TB + 128], f8)     # fp8 k^T, per-tb pad
            qT8 = kqv.tile([128, NTB, NM, 2, QB], f8)        # fp8 q^T + zeros
            v_sb = kqv.tile([128, NKT, NHL, DH + 1], bf16)   # v + ones col

            # one-time zero/one fills on gpsimd (idle engine)
            nc.gpsimd.memset(v_sb[:, :, :, DH], 1.0)
            nc.gpsimd.memset(kT8[:, :, :, TB:TB + 128], 0.0)
            nc.gpsimd.memset(qT8[:, :, :, 1, :], 0.0)

            # ---- DMA: x in halves; weight m0 chunks early, rest later ----
            wk_sb = wsb.tile([128, NDC, INNER], bf16)
            wq_sb = wsb.tile([128, NDC, INNER], bf16)
            wv_sb = wsb.tile([128, NDC, INNER], bf16)
            wo_sb = wsb.tile([128, NM, DIM], bf16)
            x_subs = []
            for tb in range(NTB):
                x_subs.append(xin.tile([128, TB // 128, DIM], bf16,
                                       name=f"x{tb}", tag="xin"))

            def dma_x(tb, half):
                sl = slice(2 * half, 2 * half + 2)
                nc.sync.dma_start(x_subs[tb][:, sl, :], x_v[tb, :, sl, :])

            def dma_xq(tb, ts):
                nc.sync.dma_start(x_subs[tb][:, ts:ts + 1, :],
                                  x_v[tb, :, ts:ts + 1, :])

            def dma_w(sb_t, view, m0, m1):
                nc.sync.dma_start(sb_t[:, :, m0 * 128:m1 * 128],
                                  view[:, :, m0 * 128:m1 * 128])

            dma_xq(0, 0)
            dma_xq(0, 1)
            dma_w(wk_sb, wk_v, 0, 1)
            dma_xq(0, 2)
            dma_xq(0, 3)
            dma_w(wq_sb, wq_v, 0, 1)
            dma_x(1, 0)
            dma_x(1, 1)
            dma_x(2, 0)
            dma_x(2, 1)
            dma_x(3, 0)
            dma_x(3, 1)
            dma_w(wv_sb, wv_v, 0, 1)
            dma_w(wk_sb, wk_v, 1, 4)
            dma_w(wv_sb, wv_v, 1, 4)
            dma_w(wq_sb, wq_v, 1, 4)
            nc.sync.dma_start(wo_sb, wo_v)

            # ---------------- projection groups ----------------
            emitted = collections.Counter()   # producer completion tracking

            def udx(h, qb):
                return (h // 2) * 8 + qb * 2 + (h % 2)

            xTs = []

            def tb_chain(tb, with_q):
                """ts-major transposes (pipelining with the x DMA quarters,
                alternating psum pools to avoid buf WAR serialization),
                followed by the m0 k (and optionally q) projection."""
                xT = xTp.tile([128, NDC, TB], bf16, name=f"xT{tb}", tag="xT")
                xTs.append(xT)
                for ts in range(TB // 128):
                    pool, tg = (ps_av, "av") if ts % 2 == 0 else (ps_f, "f")
                    pt = pool.tile([128, NDC, 128], bf16, name="pt", tag=tg)
                    for dc in range(NDC):
                        nc.tensor.transpose(
                            pt[:, dc, :],
                            x_subs[tb][:, ts, dc * 128:dc * 128 + 128],
                            ident)
                    nc.vector.tensor_copy(
                        xT[:, :, ts * 128:ts * 128 + 128], pt)
                if not with_q:
                    k_group(tb, 0)
                    emitted.update([("k", 0)])
                    return
                psk = ps_f.tile([128, TB], f32, name="pskc", tag="f")
                psq = ps_f.tile([128, TB], f32, name="psqc", tag="f")
                for dc in range(NDC):
                    nc.tensor.matmul(psk, wk_sb[:, dc, 0:128],
                                     xT[:, dc, :],
                                     start=(dc == 0), stop=(dc == NDC - 1))
                    if dc == NDC - 1:
                        nc.vector.tensor_copy(
                            kT8[:, 0, tb, 0:TB], psk)
                    nc.tensor.matmul(psq, wq_sb[:, dc, 0:128],
                                     xT[:, dc, :],
                                     start=(dc == 0), stop=(dc == NDC - 1))
                    if dc == NDC - 1:
                        nc.vector.tensor_copy(qT8[:, tb, 0, 0, :], psq)
                        emitted.update([("k", 0), ("q", 0, 0)])

            def kq_steps(w_sb, tb, m, evac):
                """Four steps of 2 matmuls; evac(psum) runs on the last.
                The psum group stays open between steps, so the pop
                machinery must not emit other ps_f tiles in between."""
                cell = {}

                def quarter(qtr):
                    if qtr == 0:
                        cell["ps"] = ps_f.tile([128, TB], f32, name="pskq",
                                               tag="f")
                    for dc in range(2 * qtr, 2 * qtr + 2):
                        nc.tensor.matmul(
                            cell["ps"], w_sb[:, dc, m * 128:m * 128 + 128],
                            xTs[tb][:, dc, :],
                            start=(dc == 0), stop=(dc == NDC - 1))
                    if qtr == 3:
                        evac(cell["ps"])
                return [(470, lambda q=q: quarter(q)) for q in range(3)] + \
                       [(1140, lambda: quarter(3))]

            def k_steps(tb, m):
                return kq_steps(
                    wk_sb, tb, m,
                    lambda ps: nc.vector.tensor_copy(
                        kT8[:, m, tb, 0:TB], ps))

            def q_steps(tb, m):
                return kq_steps(
                    wq_sb, tb, m,
                    lambda ps: nc.vector.tensor_copy(qT8[:, tb, m, 0, :],
                                                     ps))

            def k_group(tb, m):
                for _, fn in k_steps(tb, m):
                    fn()

            def q_group(tb, m):
                for _, fn in q_steps(tb, m):
                    fn()

            def v_group(tb, ts, h):
                """v for ONE head x 128 tokens; out natural [tok, 64]."""
                psv = ps_f.tile([128, DH], f32, name="psv", tag="f")
                for dc in range(NDC):
                    nc.tensor.matmul(
                        psv, xTs[tb][:, dc, ts * 128:ts * 128 + 128],
                        wv_sb[:, dc, h * DH:h * DH + DH],
                        start=(dc == 0), stop=(dc == NDC - 1))
                kt = tb * (TB // 128) + ts
                nc.vector.tensor_copy(v_sb[:, kt, h, 0:DH], psv)

            # ---------------- Phase A ----------------
            # Only tb0's chain (incl. q) is emitted up front; tb1-3 chains
            # are woven into unit 0's slots right before the score group
            # that needs them, so they never block earlier score groups in
            # the PE queue while waiting on their own x DMA.
            tb_chain(0, with_q=True)

            # ---------------- filler (deadline order) ----------------
            # NOTE: pops bound EMISSION order; a consumer emitted before its
            # producer reads garbage (deps are tracked in emission order).
            # Entries are (deadline_slot, steps); overdue entries are force-
            # emitted regardless of budget so knob tuning cannot break
            # correctness. Steps of one entry never interleave with another
            # entry (ps_f safety).
            filler = []
            fill_state = {"cur": None, "i": 0}
            def add_k(m):
                for tb in range(NTB):
                    dl = 64 * m - 14 + 4 * tb
                    steps = k_steps(tb, m) + \
                        [(0, lambda: emitted.update([("k", m)]))]
                    filler.append((dl, steps))

            def add_q(tb, m):
                dl = 8 * udx(2 * m, tb) - 2
                steps = q_steps(tb, m) + \
                    [(0, lambda: emitted.update([("q", tb, m)]))]
                filler.append((dl, steps))

            def add_v(h):
                # consumed by replay of unit udx(h, 0), emitted at that
                # unit + 1 (or +2 with startup backlog); never force before
                # unit 1 (all xT tiles exist after unit 0's woven chains)
                dl0 = 8 * udx(h, 0) + 8
                for tb in range(NTB):
                    for ts in range(TB // 128):
                        kt = tb * 4 + ts
                        filler.append(
                            (dl0 + kt // 4,
                             [(310, lambda tb=tb, ts=ts: v_group(tb, ts, h)),
                              (0, lambda: emitted.update([("v", h)]))]))

            # unit order: head pairs sharing an m-chunk, m-major:
            #   (h0,qb0),(h1,qb0),(h0,qb1),... so consecutive units mostly
            #   share projections. Emission deadlines (8 slots per unit):
            #   scores(u) need k[m], qT8[qb, m] by u's slot 0;
            #   replay(u) at unit u+1 needs that head's v by then.
            add_v(0)                      # by u1 (boosted budget)
            add_v(1)                      # by u2
            for tb in (1, 2, 3):
                add_q(tb, 0)              # by u2/u4/u6
            for m in (1, 2, 3):
                add_k(m)                  # by u8m
                add_q(0, m)
                add_v(2 * m)
                add_q(1, m)
                add_v(2 * m + 1)
                add_q(2, m)
                add_q(3, m)

            slot_now = [0]

            filler_sorted = [False]

            def pop_filler(budget):
                if not filler_sorted[0]:
                    filler.sort(key=lambda e: e[0])
                    filler_sorted[0] = True
                spent = 0.0
                while True:
                    overdue = bool(filler) and filler[0][0] <= slot_now[0]
                    if fill_state["cur"] is None:
                        if not filler or (spent >= budget and not overdue):
                            return
                        fill_state["cur"] = filler.pop(0)[1]
                        fill_state["i"] = 0
                    elif spent >= budget and not overdue:
                        return   # yield mid-entry; resume next slot
                    steps = fill_state["cur"]
                    cost, fn = steps[fill_state["i"]]
                    fn()
                    spent += cost
                    fill_state["i"] += 1
                    if fill_state["i"] >= len(steps):
                        fill_state["cur"] = None

            # ---------------- Phase B ----------------
            units = [(2 * m + hh, qb)
                     for m in range(NM) for qb in range(NQB)
                     for hh in range(2)]

            attnT = [attnp.tile([128, NM, QB], bf16, name=f"attnT{qb}",
                                tag="attnT") for qb in range(NQB)]

            def wo_unit(qb, qs, d):
                psf = ps_f.tile([128, 512], f32, name=f"psf{qs}{d}", tag="f")
                for m in range(NM):
                    nc.tensor.matmul(
                        psf, attnT[qb][:, m, qs * 128:qs * 128 + 128],
                        wo_sb[:, m, d * 512:d * 512 + 512],
                        start=(m == 0), stop=(m == NM - 1))
                osb = wo_unit.osbs.get((qb, qs))
                if osb is None:
                    osb = outp.tile([128, DIM], f32, name=f"osb{qs}",
                                    tag="osb")
                    wo_unit.osbs[(qb, qs)] = osb
                nc.vector.tensor_copy(osb[:, d * 512:d * 512 + 512], psf)
                if d == 1:
                    r0 = qb * QB + qs * 128
                    nc.sync.dma_start(out_d[r0:r0 + 128, :], osb)
                    del wo_unit.osbs[(qb, qs)]
            wo_unit.osbs = {}

            def add_wo(qb):
                for qs in range(4):
                    for d in range(2):
                        filler.append(
                            (10 ** 9,
                             [(880, lambda qs=qs, d=d: wo_unit(qb, qs, d))]))

            # last query block: m0-m2 accumulate early into a bf16 partial;
            # only the final head-pair's matmul stays in the tail
            QBL = NQB - 1
            osbP = {}

            def wo_partial(qs, d):
                psf = ps_f.tile([128, 512], f32, name=f"psfP{qs}{d}",
                                tag="f")
                for m in range(NM - 1):
                    nc.tensor.matmul(
                        psf, attnT[QBL][:, m, qs * 128:qs * 128 + 128],
                        wo_sb[:, m, d * 512:d * 512 + 512],
                        start=(m == 0), stop=(m == NM - 2))
                t = osbP.get(qs)
                if t is None:
                    t = outp.tile([128, DIM], bf16, name=f"osbP{qs}",
                                  tag="osbP", bufs=4)
                    osbP[qs] = t
                nc.vector.tensor_copy(t[:, d * 512:d * 512 + 512], psf)

            def add_wo_partial():
                for qs in range(4):
                    for d in range(2):
                        filler.append(
                            (10 ** 9,
                             [(700, lambda qs=qs, d=d: wo_partial(qs, d))]))

            # per-unit state
            ustate = {}   # u_idx -> dict(ets, psos, stages, h, qb)

            def emit_scores_exp(u_idx, g):
                h, qb = units[u_idx]
                po = (h % 2) * 64
                m = h // 2
                if u_idx > 0:
                    assert emitted[("k", m)] == NTB, (u_idx, m)
                    assert (qb, m) == (0, 0) or \
                        emitted[("q", qb, m)] == 1, (u_idx, qb, m)
                st = ustate[u_idx]
                pss = ps_s.tile([128, 2, QB], f32, name=f"pss{g}", tag="s")
                for i in range(2):
                    kt = 2 * g + i
                    c0 = (kt % 4) * 128
                    nc.tensor.matmul(
                        pss[:, i, :],
                        kT8[po:po + 64, m, kt // 4, c0:c0 + 256].rearrange(
                            "p (s f) -> p s f", s=2),
                        qT8[po:po + 64, qb, m, :, :],
                        start=True, stop=True, perf_mode=DR)
                et = etp.tile([128, 2, QB], bf16, name=f"et{g}", tag="et")
                nc.scalar.activation(out=et, in_=pss, func=AF.Exp,
                                     scale=SCALE)
                st["ets"].append(et)

            def replay_qs(u_idx, qs, pool=None):
                """attn@v for one query sub-tile of a finished unit."""
                h, qb = units[u_idx]
                assert emitted[("v", h)] == NKT, (u_idx, h)
                st = ustate[u_idx]
                pool = pool or ps_av
                pso = pool.tile([128, 512], f32, name=f"pso{qs}",
                                tag="av" if pool is ps_av else "s")
                st["psos"][qs] = pso
                for g in range(8):
                    et = st["ets"][g]
                    for i in range(2):
                        kt = 2 * g + i
                        nc.tensor.matmul(
                            pso[:, 0:DH + 1],
                            et[:, i, qs * 128:qs * 128 + 128],
                            v_sb[:, kt, h, :],
                            start=(kt == 0), stop=(kt == NKT - 1))

            def norm_qs(u_idx, qs):
                h, qb = units[u_idx]
                st = ustate[u_idx]
                pso = st["psos"][qs]
                recip = stp.tile([128, 1], f32, name=f"rc{qs}", tag="rc")
                nc.vector.reciprocal(recip, pso[:, DH:DH + 1])
                stage = stp.tile([128, DH], bf16, name=f"st{qs}", tag="st")
                nc.vector.tensor_scalar_mul(stage, pso[:, 0:DH], recip)
                st["stages"][qs] = stage

            def trans_qs(u_idx, qs):
                h, qb = units[u_idx]
                po = (h % 2) * 64
                m = h // 2
                st = ustate[u_idx]
                ptT = ps_av.tile([64, 128], bf16, name=f"ptT{qs}", tag="av")
                nc.tensor.transpose(ptT, st["stages"][qs], ident)
                nc.vector.tensor_copy(
                    attnT[qb][po:po + 64, m, qs * 128:qs * 128 + 128], ptT)

            def post_step(u_idx, g):
                """One step of a finished unit's post-processing."""
                h, qb = units[u_idx]
                if g == 0:
                    replay_qs(u_idx, 0)
                elif g == 1:
                    replay_qs(u_idx, 1)
                elif g == 2:
                    norm_qs(u_idx, 0)
                    replay_qs(u_idx, 2)
                elif g == 3:
                    norm_qs(u_idx, 1)
                    replay_qs(u_idx, 3)
                elif g == 4:
                    norm_qs(u_idx, 2)
                    trans_qs(u_idx, 0)
                elif g == 5:
                    norm_qs(u_idx, 3)
                    trans_qs(u_idx, 1)
                    trans_qs(u_idx, 2)
                elif g == 6:
                    trans_qs(u_idx, 3)
                    if h == 7 and qb != QBL:
                        add_wo(qb)
                    if h == 5 and qb == QBL:
                        add_wo_partial()
                    ustate.pop(u_idx, None)

            pending_posts = []   # (u_idx, step) queue; self-healing backlog

            def run_posts():
                steps = 2 if len(pending_posts) > o["backlog_hi"] else 1
                for _ in range(steps):
                    if not pending_posts:
                        return
                    u_i, st_i = pending_posts.pop(0)
                    post_step(u_i, st_i)

            for u_idx in range(len(units)):
                ustate[u_idx] = dict(ets=[], psos={}, stages={})
                if u_idx == 0:
                    budget = 0.0       # keep unit 0's PE path clean
                elif u_idx < 3:
                    budget = o["early_budget"]
                elif u_idx >= 24:
                    budget = o["late_budget"]
                else:
                    budget = o["fill_budget"]
                # delay unit 0's post by one unit (its deps arrive late);
                # the queue self-heals the backlog by unit 3.
                if u_idx == 2:
                    pending_posts.extend((0, st) for st in range(7))
                if u_idx >= 2:
                    pending_posts.extend((u_idx - 1, st) for st in range(7))
                for g in range(8):
                    slot_now[0] = 8 * u_idx + g
                    if u_idx == 0 and g in (2, 4, 6):
                        tb_chain(g // 2, with_q=False)
                    emit_scores_exp(u_idx, g)
                    pop_filler(budget)
                    run_posts()

            # ---------------- tail (pipelined per query sub-tile) ----------
            slot_now[0] = 10 ** 9 - 1
            while pending_posts:
                u_i, st_i = pending_posts.pop(0)
                post_step(u_i, st_i)
            while filler or fill_state["cur"] is not None:
                pop_filler(1e9)
            last = len(units) - 1
            replay_qs(last, 0)
            replay_qs(last, 1)
            replay_qs(last, 2, pool=ps_s)
            replay_qs(last, 3, pool=ps_s)
            for qs in range(4):
                norm_qs(last, qs)
                trans_qs(last, qs)
                for d in range(2):
                    psf = ps_f.tile([128, 512], f32, name=f"psfL{qs}{d}",
                                    tag="f")
                    nc.tensor.matmul(
                        psf, attnT[QBL][:, NM - 1, qs * 128:qs * 128 + 128],
                        wo_sb[:, NM - 1, d * 512:d * 512 + 512],
                        start=True, stop=True)
                    fin = outp.tile([128, 512], f32, name=f"fin{qs}{d}",
                                    tag="fin", bufs=4)
                    nc.vector.tensor_add(fin, osbP[qs][:, d * 512:d * 512 + 512],
                                   psf)
                    r0 = QBL * QB + qs * 128
                    nc.sync.dma_start(
                        out_d[r0:r0 + 128, d * 512:d * 512 + 512], fin)

    nc.compile()
    return nc


_NC = None


def _get_nc():
    global _NC
    if _NC is None:
        _NC = build_nc()
    return _NC


def kernel(x, Wq, Wk, Wv, Wo, bo):
    x = np.asarray(x, dtype=np.float32)
    Wq = np.asarray(Wq, dtype=np.float32)
    Wk = np.asarray(Wk, dtype=np.float32)
    Wv = np.asarray(Wv, dtype=np.float32)
    Wo = np.asarray(Wo, dtype=np.float32)
    bo = np.asarray(bo, dtype=np.float32)

    B = x.shape[0]
    bf = ml_dtypes.bfloat16
    nc = _get_nc()
    in_maps = []
    for c in range(8):
        b, hh = c // 2, c % 2
        sl = slice(hh * INNER, hh * INNER + INNER)
        in_maps.append({
            "x": np.ascontiguousarray(x[b].astype(bf)),
            "wq": np.ascontiguousarray(Wq[:, sl].astype(bf)),
            "wk": np.ascontiguousarray(Wk[:, sl].astype(bf)),
            "wv": np.ascontiguousarray(Wv[:, sl].astype(bf)),
            "wo": np.ascontiguousarray(Wo[sl, :].astype(bf)),
        })
    res = run_bass_kernel_spmd(nc, in_maps, core_ids=list(range(8)))
    out = np.empty((B, N, DIM), dtype=np.float32)
    for b in range(B):
        out[b] = res.results[2 * b]["out"] + res.results[2 * b + 1]["out"] + bo
    return out
